# revision 20
# baseline (speedup 1.0000x reference)
"""Trainium2 Bass kernel for BilinearInteraction (v3).

out[b,p,:] = (x[:,i,:] @ W_p) * x[:,j,:] for the P=276 pairs (i,j)=comb(24,2),
B=2048, E=128.

Sharding: 4 batch-quarters x 2 pair-halves on 8 cores (one SPMD program).

Pair split (program-identical across the two halves):
  - Fields are paired (2k+1, 2k).  "Main run" k covers second-fields
    j in [2k+2, 24): half A uses stationary field 2k+1, half B field 2k.
    Same (window offset, length) for both halves, so one program; only the
    per-core xT / W contents and the host-side output permutation differ.
  - The 12 leftover "adjacent" pairs (2k, 2k+1) are computed by BOTH halves,
    each for half of the batch quarter (host maps program-bc chunks to
    different physical rows per half).

Per-core DMA (bf16): W 4.72 MB + xT 2.23 MB + xn 3.01 MB + out 18.1 MB
= 28.1 MB (vs 30.3 MB for the 8-way batch-parallel v1).

PSUM drain (the v2 lesson: DVE reads PSUM at only ~106 G elem/s, so an
all-DVE drain serializes the kernel): each 1024-col PSUM tile is split
  - direct  [0:384):   DVE tensor_mul straight from PSUM (fp32 in, bf16 out)
  - copied  [384:end): one ScalarE copy to SBUF bf16, then bf16 tensor_mul
                       on GpSimd (60%) or DVE (40%)
which lands DVE/ACT/GPS each at ~45 us, under the ~67 us DMA roof.
Stores are merged to ~1 MB spans (4 PSUM tiles per store).
"""

import numpy as np
import ml_dtypes

# ---------------------------------------------------------------- constants
F = 24
E = 128
B = 2048
P = F * (F - 1) // 2
NCORES = 8
NQ = 4                      # batch quarters
B_LOCAL = B // NQ           # 512 rows per core
BCH = B_LOCAL // 128        # 4 chunks of 128

# main runs k=0..10: stationary field (h=0: 2k+1, h=1: 2k), seconds [2k+2,24)
NRUNS = 11
RLEN = [22 - 2 * k for k in range(NRUNS)]            # pairs per run
RS = [0]
for l in RLEN:
    RS.append(RS[-1] + l * E)                        # run col starts
MAIN_COLS = RS[-1]                                   # 16896
NADJ = 12                                            # adjacent pairs (2k,2k+1)
ADJ_COLS = NADJ * E                                  # 1536
WT_COLS = MAIN_COLS + ADJ_COLS                       # 18432

XT_MAIN = NRUNS * B_LOCAL                            # 5632
XT_COLS = XT_MAIN + NADJ * (B_LOCAL // 2)            # 8704
XN_F = F - 1                                         # fields 1..23
XN_BC = XN_F * E                                     # 2944 cols per bc chunk
XN_COLS = BCH * XN_BC                                # 11776

MM_MAX = 512                                         # max matmul free dim
BANK = 512                                           # fp32 elems per PSUM bank
PSUM_TILE = 1024                                     # 2 banks, 4 bufs
STORE_SPAN = 4096                                    # cols per output store
DIRECT_FRAC = 0.375                                  # DVE-direct share of cols
GPS_FRAC = 0.32                                      # GpSimd share of copied TTs

DVE, GPS = 0, 1


def _runk(col):
    return next(k for k in range(NRUNS) if RS[k] <= col < RS[k + 1])


def _cut_runs(lo, hi):
    """Split [lo,hi) of main cols at run boundaries -> [(lo,hi,runk)]."""
    out = []
    while lo < hi:
        k = _runk(lo)
        nxt = min(RS[k + 1], hi)
        out.append((lo, nxt, k))
        lo = nxt
    return out


def _main_schedule():
    """Per-bc schedule: store spans, each with psum tiles, segs and drains."""
    spans = []
    gps_acc = 0.0
    s0 = 0
    while s0 < MAIN_COLS:
        scols = min(STORE_SPAN, MAIN_COLS - s0)
        tiles = []
        t0 = s0
        while t0 < s0 + scols:
            cols = min(PSUM_TILE, s0 + scols - t0)
            cuts = set(range(t0, t0 + cols, BANK))
            for k in range(NRUNS):
                if t0 < RS[k] < t0 + cols:
                    cuts.add(RS[k])
            cuts = sorted(cuts) + [t0 + cols]
            segs = []
            for a, b in zip(cuts, cuts[1:]):
                segs.append([a, b - a, _runk(a), (a - t0) // BANK])
                assert (a - t0 + (b - a) - 1) // BANK == segs[-1][3]
            out_segs = []
            for s, (a, n, k, span) in enumerate(segs):
                first = s == 0 or segs[s - 1][3] != span
                last = s == len(segs) - 1 or segs[s + 1][3] != span
                out_segs.append((a, n, k, first, last))
            dcols = int(cols * DIRECT_FRAC) // 128 * 128
            direct = _cut_runs(t0, t0 + dcols)
            copied = _cut_runs(t0 + dcols, t0 + cols)
            gps_acc += GPS_FRAC
            eng = GPS if gps_acc >= 1.0 else DVE
            if eng == GPS:
                gps_acc -= 1.0
            tiles.append(dict(t0=t0, cols=cols, segs=out_segs, dcols=dcols,
                              direct=direct, copied=copied, eng=eng))
            t0 += cols
        spans.append(dict(s0=s0, scols=scols, tiles=tiles))
        s0 += scols
    return spans


MSPANS = _main_schedule()

_NC = None


def _build_module():
    global _NC
    if _NC is not None:
        return _NC

    import concourse.bass as bass
    import concourse.tile as tile
    from concourse import bacc, mybir

    bf = mybir.dt.bfloat16
    f32 = mybir.dt.float32

    nc = bacc.Bacc("TRN2", target_bir_lowering=False, debug=False)

    xT = nc.declare_dram_parameter("xT", [128, XT_COLS], bf, isOutput=False)
    xn = nc.declare_dram_parameter("xn", [128, XN_COLS], bf, isOutput=False)
    Wt = nc.declare_dram_parameter("Wt", [128, WT_COLS], bf, isOutput=False)
    out = nc.declare_dram_parameter("out", [B_LOCAL, MAIN_COLS], bf,
                                    isOutput=True)
    outd = nc.declare_dram_parameter("outd", [B_LOCAL // 2, ADJ_COLS], bf,
                                     isOutput=True)

    with tile.TileContext(nc) as tc:
        with (
            tc.tile_pool(name="const", bufs=1) as cpool,
            tc.tile_pool(name="mm", bufs=4) as mmpool,
            tc.tile_pool(name="so", bufs=6) as sopool,
            tc.tile_pool(name="ps", bufs=4, space=bass.MemorySpace.PSUM) as pspool,
        ):
            xT_sb = cpool.tile([128, XT_COLS], bf, tag="xT")
            xn_sb = cpool.tile([128, XN_COLS], bf, tag="xn")
            w_sb = cpool.tile([128, WT_COLS], bf, tag="w")

            # all loads up front on the single ring; loads (10 MB) finish
            # around when the first stores become ready.
            nc.scalar.dma_start(out=xT_sb[:, 0:1536], in_=xT[:, 0:1536])
            nc.scalar.dma_start(out=w_sb[:, 0:2048], in_=Wt[:, 0:2048])
            nc.scalar.dma_start(out=xT_sb[:, 1536:XT_COLS], in_=xT[:, 1536:XT_COLS])
            nc.scalar.dma_start(out=w_sb[:, 2048:6144], in_=Wt[:, 2048:6144])
            nc.scalar.dma_start(out=xn_sb[:, 0:XN_BC], in_=xn[:, 0:XN_BC])
            nc.scalar.dma_start(out=w_sb[:, 6144:10240], in_=Wt[:, 6144:10240])
            nc.scalar.dma_start(out=xn_sb[:, XN_BC:XN_COLS], in_=xn[:, XN_BC:XN_COLS])
            nc.scalar.dma_start(out=w_sb[:, 10240:MAIN_COLS], in_=Wt[:, 10240:MAIN_COLS])
            nc.scalar.dma_start(out=w_sb[:, MAIN_COLS:WT_COLS], in_=Wt[:, MAIN_COLS:WT_COLS])

            def stationary(k, bc):
                o = k * B_LOCAL + bc * 128
                return xT_sb[:, o: o + 128]

            def adj_stationary(k, bc):
                o = XT_MAIN + k * 256 + bc * 128
                return xT_sb[:, o: o + 128]

            def xnw(bc, k, lo):
                # multiplier window for run k starts at field 2k+2
                return bc * XN_BC + (2 * k + 1) * E + (lo - RS[k])

            # one explicit LDWEIGHTS per stationary change; matmuls are
            # marked non-self-loading (the stationary xT slice persists in
            # the PE array across consecutive matmuls of the same run).
            cur_st = [None]

            def mm_ls(out_ap, st_key, st_ap, rhs_ap, start, stop):
                if cur_st[0] != st_key:
                    nc.tensor.ldweights(st_ap)
                    cur_st[0] = st_key
                inst = nc.tensor.matmul(out_ap, st_ap, rhs_ap,
                                        start=start, stop=stop)
                inst.ins.ldweights = False
                return inst

            # copied-multiplies are deferred by one psum tile so DVE's FIFO
            # never head-of-line blocks on an in-flight ACT copy: when DVE
            # reaches tile t's multiplies, the copy has long finished.
            pending = []

            def flush_pending():
                while pending:
                    pending.pop(0)()

            for bc in range(BCH):
                for sp in MSPANS:
                    so = sopool.tile([128, sp["scols"]], bf, tag="so")
                    for t in sp["tiles"]:
                        t0, cols = t["t0"], t["cols"]
                        ps = pspool.tile([128, cols], f32, tag="ps")
                        for (a, n, k, first, last) in t["segs"]:
                            mm_ls(ps[:, a - t0: a - t0 + n],
                               ("m", k, bc), stationary(k, bc),
                               w_sb[:, a: a + n], first, last)
                        s0 = sp["s0"]
                        for (lo, hi, k) in t["direct"]:
                            nc.vector.tensor_mul(
                                so[:, lo - s0: hi - s0],
                                ps[:, lo - t0: hi - t0],
                                xn_sb[:, xnw(bc, k, lo): xnw(bc, k, lo) + hi - lo],
                            )
                        ccols = cols - t["dcols"]
                        if ccols:
                            mm = mmpool.tile([128, ccols], bf, tag="mm")
                            c0 = t0 + t["dcols"]
                            nc.scalar.copy(out=mm[:], in_=ps[:, c0 - t0: cols])
                            emul = nc.gpsimd if t["eng"] == GPS else nc.vector

                            def mul_t(emul=emul, mm=mm, so=so, t=t, s0=s0,
                                      c0=c0, bc=bc):
                                for (lo, hi, k) in t["copied"]:
                                    emul.tensor_mul(
                                        so[:, lo - s0: hi - s0],
                                        mm[:, lo - c0: hi - c0],
                                        xn_sb[:, xnw(bc, k, lo):
                                              xnw(bc, k, lo) + hi - lo],
                                    )
                            flush_pending()
                            pending.append(mul_t)

                    def store_sp(so=so, sp=sp, bc=bc):
                        nc.sync.dma_start(
                            out=out[bc * 128: bc * 128 + 128,
                                    sp["s0"]: sp["s0"] + sp["scols"]],
                            in_=so[:],
                        )
                    pending.append(store_sp)
                # adjacent pairs on program-bc 0,1 only
                if bc < 2:
                    so = sopool.tile([128, ADJ_COLS], bf, tag="so")
                    for (p0, pcols) in ((0, 1024), (1024, 512)):
                        ps = pspool.tile([128, pcols], f32, tag="ps")
                        for k in range(p0 // E, (p0 + pcols) // E):
                            a = k * E - p0
                            first = a % BANK == 0
                            last = (a + E) % BANK == 0 or k == NADJ - 1
                            mm_ls(ps[:, a: a + E],
                               ("a", k, bc), adj_stationary(k, bc),
                               w_sb[:, MAIN_COLS + k * E: MAIN_COLS + k * E + E],
                               first, last)
                        nb = pcols // E

                        def mul_adj(ps=ps, so=so, p0=p0, nb=nb, bc=bc):
                            # one strided TT over nb pair blocks: multiplier
                            # fields 2k+1 live at xn offsets 2k*E (stride 2E)
                            xo = bc * XN_BC + 2 * (p0 // E) * E
                            xv = xn_sb[:, xo: xo + 2 * nb * E].rearrange(
                                "p (k t e) -> p k t e", k=nb, t=2, e=E)[:, :, 0, :]
                            nc.vector.tensor_mul(
                                so[:, p0: p0 + nb * E].rearrange(
                                    "p (k e) -> p k e", k=nb, e=E),
                                ps[:].rearrange("p (k e) -> p k e", k=nb, e=E),
                                xv,
                            )
                        flush_pending()
                        pending.append(mul_adj)

                    def store_adj(so=so, bc=bc):
                        nc.sync.dma_start(
                            out=outd[bc * 128: bc * 128 + 128, :],
                            in_=so[:],
                        )
                    pending.append(store_adj)
            flush_pending()

    nc.compile()
    _NC = nc
    return nc


def _pair_index(i, j):
    return i * (2 * F - i - 1) // 2 + (j - i - 1)


def _prep_inputs(x, W):
    """Host-side shard + relayout + bf16 cast. Returns in_maps for 8 cores."""
    bf = ml_dtypes.bfloat16
    x = np.ascontiguousarray(x, dtype=np.float32)
    W = np.ascontiguousarray(W, dtype=np.float32)

    Wt_h = []
    for h in (0, 1):
        plist = []
        for k in range(NRUNS):
            i = 2 * k + 1 - h
            for j in range(2 * k + 2, F):
                plist.append(_pair_index(i, j))
        for k in range(NADJ):
            plist.append(_pair_index(2 * k, 2 * k + 1))
        Wm = W[np.array(plist, dtype=np.int64)]          # [144,128,128]
        Wt_h.append(np.ascontiguousarray(
            Wm.transpose(1, 0, 2).reshape(128, WT_COLS)).astype(bf))

    in_maps = []
    for c in range(NCORES):
        q, h = c >> 1, c & 1
        xs = x[q * B_LOCAL: (q + 1) * B_LOCAL]           # [512,24,128]
        perm = np.concatenate(
            [np.arange(128) + ((bc + 2 * h) % 4) * 128 for bc in range(BCH)])
        xsp = xs[perm]                                   # bc-permuted rows
        fst = np.array([2 * k + 1 - h for k in range(NRUNS)])
        xT_main = xsp[:, fst, :].transpose(2, 1, 0).reshape(128, XT_MAIN)
        xT_adj = xsp[:256, np.arange(0, F, 2), :].transpose(2, 1, 0).reshape(
            128, NADJ * 256)
        xTv = np.ascontiguousarray(
            np.concatenate([xT_main, xT_adj], axis=1)).astype(bf)
        xnv = xsp.reshape(BCH, 128, F, E)[:, :, 1:, :].transpose(
            1, 0, 2, 3).reshape(128, XN_COLS)
        xnv = np.ascontiguousarray(xnv).astype(bf)
        in_maps.append({"xT": xTv, "xn": xnv, "Wt": Wt_h[h]})
    return in_maps


def _unshard(results):
    out_full = np.empty((B, P, E), dtype=np.float32)
    pmap_h = []
    padj = np.array([_pair_index(2 * k, 2 * k + 1) for k in range(NADJ)])
    for h in (0, 1):
        pl = []
        for k in range(NRUNS):
            i = 2 * k + 1 - h
            for j in range(2 * k + 2, F):
                pl.append(_pair_index(i, j))
        pmap_h.append(np.array(pl, dtype=np.int64))
    for c in range(NCORES):
        q, h = c >> 1, c & 1
        grows = q * B_LOCAL + np.concatenate(
            [np.arange(128) + ((bc + 2 * h) % 4) * 128 for bc in range(BCH)])
        o = np.asarray(results[c]["out"]).astype(np.float32).reshape(
            B_LOCAL, MAIN_COLS // E, E)
        out_full[grows[:, None], pmap_h[h][None, :], :] = o
        od = np.asarray(results[c]["outd"]).astype(np.float32).reshape(
            B_LOCAL // 2, NADJ, E)
        out_full[grows[:256, None], padj[None, :], :] = od
    return out_full


def run_on_hw(x, W, trace=False, **run_kwargs):
    from concourse.bass_utils import run_bass_kernel_spmd

    nc = _build_module()
    in_maps = _prep_inputs(x, W)
    res = run_bass_kernel_spmd(
        nc, in_maps, list(range(NCORES)), trace=trace, **run_kwargs
    )
    return _unshard(res.results), res


def kernel(x, W):
    import os
    try:
        out, _ = run_on_hw(x, W, trace=False)
    except Exception:
        os.environ["NEURON_RT_RESET_CORES"] = "1"
        out, _ = run_on_hw(x, W, trace=False)
    return out
